# revision 3
# baseline (speedup 1.0000x reference)
"""KGram MLP seq model (k-gram embedding lookup + 2-layer MLP + vocab projection)
on 8 Trainium2 NeuronCores.

Strategy: data-parallel over the S*B = 4096 token positions (512 rows/core,
cores 0-3 take batch 0, cores 4-7 take batch 1; each core owns a contiguous
span of 512 sequence positions of one batch column).  All weights are
replicated per core (uploaded as bf16).  Per core:

  1. ONE indirect-DMA gather of the 640 (padded) embedding rows from E
     (multi-column offset AP -> [128, NG, D] token-major tile)
  2. bounce token-major tile to a DRAM scratch [640, D], then ONE big
     DRAM->SBUF xbar transpose into the feature-major layout
     GT[p, f, t] = E[tok[t], f*128+p]  ([128, DK, TWPAD] tile)
  3. h1^T = silu(W1^T x^T + b1) where the three K-blocks of x^T are just
     shifted column windows of GT (the k-gram windows overlap)
  4. h2^T = silu(W2^T h1^T + b2)
  5. logits^T = Wout^T h2^T + bout, streamed over vocab in 1024-col groups
     (bf16 weights, f32 PSUM accumulate, bf16 output store; host upcasts)

PE warmup matmuls cover the gather/transpose window so the HAM clock never
drops; all heavy DMA (Wout in, logits out) is split across the two HWDGE
rings (sync: weights in, scalar: logits out).

Host reassembles out[s, b, :] from the per-core logits^T shards.
"""

import math

import numpy as np
import ml_dtypes

import concourse.bass as bass
import concourse.mybir as mybir
import concourse.tile as tile
from concourse import bacc
from concourse.bass_utils import run_bass_kernel_spmd

P = 128
NCORES = 8

# Full-problem constants (hardcoded; kernel.py must be self-contained)
VOCAB = 50257
EMBED = 1024
SEQ = 2048
BATCH = 2
KGRAM = 3
VPAD = 50304  # 393 * 128
MGROUP = 1024  # vocab columns per Wout streaming group

N_WARM = 85       # FD-512 warmup matmuls covering the gather/transpose window
SPLIT_GATHER = True  # fallback: one indirect DMA per token-group column

_nc_cache: dict = {}


def _build(V, D, KC, T, VP, MG):
    """Build the single-core Bass graph (SPMD: same graph on all cores)."""
    DK = D // P
    TW = T + KC - 1
    NG = math.ceil(TW / P)
    TWPAD = NG * P
    NM = VP // P
    f32 = mybir.dt.float32
    bf16 = mybir.dt.bfloat16
    i32 = mybir.dt.int32
    AF = mybir.ActivationFunctionType

    nc = bacc.Bacc()

    E_d = nc.declare_dram_parameter("E", [V, D], bf16, isOutput=False)
    W1_d = nc.declare_dram_parameter("W1", [KC * D, D], bf16, isOutput=False)
    W2_d = nc.declare_dram_parameter("W2", [D, D], bf16, isOutput=False)
    Wo_d = nc.declare_dram_parameter("Wo", [D, VP], bf16, isOutput=False)
    b1_d = nc.declare_dram_parameter("b1", [P, DK], f32, isOutput=False)
    b2_d = nc.declare_dram_parameter("b2", [P, DK], f32, isOutput=False)
    bo_d = nc.declare_dram_parameter("bo", [P, NM], f32, isOutput=False)
    tok_d = nc.declare_dram_parameter("toks", [P, NG], i32, isOutput=False)
    out_d = nc.declare_dram_parameter("out", [VP, T], bf16, isOutput=True)

    with tile.TileContext(nc) as tc:
        with (
            tc.tile_pool(name="const", bufs=1) as cpool,
            tc.tile_pool(name="gath", bufs=1) as gpool,
            tc.tile_pool(name="gt", bufs=1) as gtpool,
            tc.tile_pool(name="dram", bufs=1, space="DRAM") as dpool,
            tc.tile_pool(name="w", bufs=1) as wpool,
            tc.tile_pool(name="h", bufs=1) as hpool,
            tc.tile_pool(name="wo", bufs=2) as wopool,
            tc.tile_pool(name="ot", bufs=4) as opool,
            tc.tile_pool(name="psA", bufs=2, space="PSUM") as psA,
            tc.tile_pool(name="psB", bufs=6, space="PSUM") as psB,
        ):
            # token indices first so the gather starts immediately
            tok_s = cpool.tile([P, NG], i32, tag="tok")
            nc.sync.dma_start(tok_s[:], tok_d[:])

            # --- embedding gather (token-major): G[p, g, :] = E[tok[p, g], :]
            G = gpool.tile([P, NG, D], bf16, tag="g", name="g")
            if SPLIT_GATHER:
                for g in range(NG):
                    nc.gpsimd.indirect_dma_start(
                        out=G[:, g, :],
                        out_offset=None,
                        in_=E_d[:],
                        in_offset=bass.IndirectOffsetOnAxis(
                            ap=tok_s[:, g : g + 1], axis=0
                        ),
                    )
            else:
                nc.gpsimd.indirect_dma_start(
                    out=G[:],
                    out_offset=None,
                    in_=E_d[:],
                    in_offset=bass.IndirectOffsetOnAxis(ap=tok_s[:], axis=0),
                )

            # --- bounce to DRAM in token-row-major, then one xbar transpose
            # back to the feature-major GT[p, f, t] = E[tok[t], f*128+p]
            scratch = dpool.tile([TWPAD, D], bf16, tag="scr", name="scr")
            nc.scalar.dma_start(
                scratch.rearrange("(g p) d -> p g d", p=P), G[:]
            )
            GT = gtpool.tile([P, DK, TWPAD], bf16, tag="gt", name="gt")
            nc.scalar.dma_start_transpose(GT[:], scratch[:])

            # PE warmup: burn the HAM cold window on dummy matmuls while the
            # gather/transpose pipeline is in flight, so the real matmul
            # stream starts at full clock with no idle gap.
            warm = cpool.tile([P, T], bf16, tag="warm")
            nc.vector.memset(warm[:], 0.5)
            warm_ps = psA.tile([P, T], f32, tag="mlp", name="warm_ps")
            for _ in range(N_WARM):
                nc.tensor.matmul(
                    warm_ps[:], lhsT=warm[:, :P], rhs=warm[:],
                    start=True, stop=True,
                )

            b1_s = cpool.tile([P, DK], f32, tag="b1")
            nc.sync.dma_start(b1_s[:], b1_d[:])
            b2_s = cpool.tile([P, DK], f32, tag="b2")
            nc.sync.dma_start(b2_s[:], b2_d[:])
            bo_s = cpool.tile([P, NM], f32, tag="bo")
            nc.sync.dma_start(bo_s[:], bo_d[:])

            # --- MLP layer 1: h1^T = silu(W1^T x^T + b1) ---
            w1s = []
            for kc in range(KC * DK):
                t = wpool.tile([P, D], bf16, tag=f"w1_{kc}", name=f"w1_{kc}")
                nc.sync.dma_start(t[:], W1_d[kc * P : (kc + 1) * P, :])
                w1s.append(t)
            h1 = [hpool.tile([P, T], bf16, tag=f"h1_{m}", name=f"h1_{m}") for m in range(DK)]
            for m in range(DK):
                ps = psA.tile([P, T], f32, tag="mlp")
                n = 0
                for i in range(KC):
                    for k8 in range(DK):
                        kc = i * DK + k8
                        nc.tensor.matmul(
                            ps[:],
                            lhsT=w1s[kc][:, m * P : (m + 1) * P],
                            rhs=GT[:, k8, i : i + T],
                            start=(n == 0),
                            stop=(n == KC * DK - 1),
                        )
                        n += 1
                nc.scalar.activation(h1[m][:], ps[:], AF.Silu, bias=b1_s[:, m : m + 1])

            # --- MLP layer 2: h2^T = silu(W2^T h1^T + b2) ---
            w2s = []
            for kc in range(DK):
                t = wpool.tile([P, D], bf16, tag=f"w2_{kc}", name=f"w2_{kc}")
                nc.sync.dma_start(t[:], W2_d[kc * P : (kc + 1) * P, :])
                w2s.append(t)
            h2 = [hpool.tile([P, T], bf16, tag=f"h2_{m}", name=f"h2_{m}") for m in range(DK)]
            for m in range(DK):
                ps = psA.tile([P, T], f32, tag="mlp")
                for k8 in range(DK):
                    nc.tensor.matmul(
                        ps[:],
                        lhsT=w2s[k8][:, m * P : (m + 1) * P],
                        rhs=h1[k8][:],
                        start=(k8 == 0),
                        stop=(k8 == DK - 1),
                    )
                nc.scalar.activation(h2[m][:], ps[:], AF.Silu, bias=b2_s[:, m : m + 1])

            # --- vocab projection: logits^T = Wout^T h2^T + bout ---
            c0 = 0
            while c0 < VP:
                cols = min(MG, VP - c0)
                wos = []
                for k8 in range(DK):
                    t = wopool.tile([P, MG], bf16, tag=f"wo{k8}", name=f"wo{k8}")
                    nc.sync.dma_start(
                        t[:, :cols], Wo_d[k8 * P : (k8 + 1) * P, c0 : c0 + cols]
                    )
                    wos.append(t)
                for m in range(cols // P):
                    ps = psB.tile([P, T], f32, tag="proj")
                    for k8 in range(DK):
                        nc.tensor.matmul(
                            ps[:],
                            lhsT=wos[k8][:, m * P : (m + 1) * P],
                            rhs=h2[k8][:],
                            start=(k8 == 0),
                            stop=(k8 == DK - 1),
                        )
                    ot = opool.tile([P, T], bf16, tag="ot")
                    mi = (c0 + m * P) // P
                    nc.scalar.activation(
                        ot[:], ps[:], AF.Identity, bias=bo_s[:, mi : mi + 1]
                    )
                    nc.scalar.dma_start(out_d[c0 + m * P : c0 + (m + 1) * P, :], ot[:])
                c0 += cols

    nc.finalize()
    return nc


def _get_nc(V, D, KC, T, VP, MG):
    key = (V, D, KC, T, VP, MG)
    if key not in _nc_cache:
        _nc_cache[key] = _build(V, D, KC, T, VP, MG)
    return _nc_cache[key]


def _run(tokens, E, W1, b1, W2, b2, Wout, bout, V, D, KC, VP, MG, trace=False):
    """tokens: (S, B) int32.  Returns (S, B, V) f32 logits (and results obj)."""
    bf16 = ml_dtypes.bfloat16
    S, B = tokens.shape
    cpb = NCORES // B  # cores per batch column
    T = S // cpb
    DK = D // P
    TW = T + KC - 1
    NG = math.ceil(TW / P)
    TWPAD = NG * P
    NM = VP // P

    E_b = E.astype(bf16)
    W1_b = W1.astype(bf16)
    W2_b = W2.astype(bf16)
    Wo_b = np.zeros((D, VP), dtype=bf16)
    Wo_b[:, :V] = Wout.astype(bf16)
    b1t = np.ascontiguousarray(b1.reshape(DK, P).T.astype(np.float32))
    b2t = np.ascontiguousarray(b2.reshape(DK, P).T.astype(np.float32))
    bo_p = np.zeros(VP, dtype=np.float32)
    bo_p[:V] = bout
    bot = np.ascontiguousarray(bo_p.reshape(NM, P).T)

    nc = _get_nc(V, D, KC, T, VP, MG)

    in_maps = []
    for c in range(NCORES):
        b, chunk = divmod(c, cpb)
        s0 = chunk * T
        pad = np.zeros(TWPAD, dtype=np.int32)
        lo = max(0, s0 - (KC - 1))
        seg = tokens[lo : s0 + T, b]
        start = (KC - 1) - (s0 - lo)
        pad[start : start + seg.size] = seg
        tok2d = np.ascontiguousarray(pad.reshape(NG, P).T)
        in_maps.append(
            {
                "E": E_b,
                "W1": W1_b,
                "W2": W2_b,
                "Wo": Wo_b,
                "b1": b1t,
                "b2": b2t,
                "bo": bot,
                "toks": tok2d,
            }
        )

    kres = run_bass_kernel_spmd(nc, in_maps, list(range(NCORES)), trace=trace)
    res = kres.results

    out = np.empty((S, B, V), dtype=np.float32)
    for c in range(NCORES):
        b, chunk = divmod(c, cpb)
        s0 = chunk * T
        out[s0 : s0 + T, b, :] = res[c]["out"][:V, :].T.astype(np.float32)
    return out, kres


def kernel(**inputs):
    tokens = np.asarray(inputs["tokens_seq"]).astype(np.int32)
    E = np.asarray(inputs["E"], dtype=np.float32)
    W1 = np.asarray(inputs["W1"], dtype=np.float32)
    b1 = np.asarray(inputs["b1"], dtype=np.float32)
    W2 = np.asarray(inputs["W2"], dtype=np.float32)
    b2 = np.asarray(inputs["b2"], dtype=np.float32)
    Wout = np.asarray(inputs["Wout"], dtype=np.float32)
    bout = np.asarray(inputs["bout"], dtype=np.float32)
    out, _ = _run(
        tokens, E, W1, b1, W2, b2, Wout, bout,
        V=VOCAB, D=EMBED, KC=KGRAM, VP=VPAD, MG=MGROUP,
    )
    return out


# revision 6
# speedup vs baseline: 1.0042x; 1.0042x over previous
"""KGram MLP seq model (k-gram embedding lookup + 2-layer MLP + vocab projection)
on 8 Trainium2 NeuronCores.

Strategy: data-parallel over the S*B = 4096 token positions (512 rows/core,
cores 0-3 take batch 0, cores 4-7 take batch 1; each core owns a contiguous
span of 512 sequence positions of one batch column).  All weights are
replicated per core (uploaded as bf16).  Per core:

  1. indirect-DMA gather of the 640 (padded) embedding rows from E
     into a token-major [128, NG, D] tile (one DMA per 128-token group)
  2. bounce each group to a DRAM scratch [640, D] (row-major by position),
     then two big DRAM->SBUF xbar transposes into the feature-major layout
     GT[p, f, t] = E[tok[t], f*128+p]  ([128, DK, TWPAD] tile)
  3. h1^T = silu(W1^T x^T + b1) where the three K-blocks of x^T are just
     shifted column windows of GT (the k-gram windows overlap); k-outer
     loop accumulates all 8 output blocks across the 8 PSUM banks
  4. h2^T = silu(W2^T h1^T + b2)
  5. logits^T = Wout^T h2^T + bout, streamed over vocab in 1024-col groups
     (bf16 weights, f32 PSUM accumulate, bf16 output store; host upcasts)

PE warmup matmuls cover the gather/transpose window so the HAM clock never
drops; heavy DMA is split across the two HWDGE rings (sync: weights in,
scalar: logits out) with batched transfers (one DMA per weight group, one
store per 4 output tiles).

Host reassembles out[s, b, :] from the per-core logits^T shards.
"""

import math

import numpy as np
import ml_dtypes

import concourse.bass as bass
import concourse.mybir as mybir
import concourse.tile as tile
from concourse import bacc
from concourse.bass_utils import run_bass_kernel_spmd

P = 128
NCORES = 8

# Full-problem constants (hardcoded; kernel.py must be self-contained)
VOCAB = 50257
EMBED = 1024
SEQ = 2048
BATCH = 2
KGRAM = 3
VPAD = 50304  # 393 * 128
MGROUP = 1024  # vocab columns per Wout streaming group
SBATCH = 4    # output tiles per store DMA
N_WARM = 135  # FD-512 warmup matmuls covering the gather/transpose window

_nc_cache: dict = {}


def _build(V, D, KC, T, VP, MG):
    """Build the single-core Bass graph (SPMD: same graph on all cores)."""
    DK = D // P
    TW = T + KC - 1
    NG = math.ceil(TW / P)
    TWPAD = NG * P
    NM = VP // P
    f32 = mybir.dt.float32
    bf16 = mybir.dt.bfloat16
    i32 = mybir.dt.int32
    AF = mybir.ActivationFunctionType

    nc = bacc.Bacc()

    E_d = nc.declare_dram_parameter("E", [V, D], bf16, isOutput=False)
    W1_d = nc.declare_dram_parameter("W1", [KC * D, D], bf16, isOutput=False)
    W2_d = nc.declare_dram_parameter("W2", [D, D], bf16, isOutput=False)
    Wo_d = nc.declare_dram_parameter("Wo", [D, VP], bf16, isOutput=False)
    b1_d = nc.declare_dram_parameter("b1", [P, DK], f32, isOutput=False)
    b2_d = nc.declare_dram_parameter("b2", [P, DK], f32, isOutput=False)
    bo_d = nc.declare_dram_parameter("bo", [P, NM], f32, isOutput=False)
    tok_d = nc.declare_dram_parameter("toks", [P, NG], i32, isOutput=False)
    out_d = nc.declare_dram_parameter("out", [VP, T], bf16, isOutput=True)

    with tile.TileContext(nc) as tc:
        with (
            tc.tile_pool(name="const", bufs=1) as cpool,
            tc.tile_pool(name="gath", bufs=1) as gpool,
            tc.tile_pool(name="gt", bufs=1) as gtpool,
            tc.tile_pool(name="dram", bufs=1, space="DRAM") as dpool,
            tc.tile_pool(name="w", bufs=1) as wpool,
            tc.tile_pool(name="h", bufs=1) as hpool,
            tc.tile_pool(name="wo", bufs=2) as wopool,
            tc.tile_pool(name="ot", bufs=4) as opool,
            tc.tile_pool(name="ps", bufs=8, space="PSUM") as pspool,
        ):
            # token indices first so the gather starts immediately
            tok_s = cpool.tile([P, NG], i32, tag="tok")
            nc.sync.dma_start(tok_s[:], tok_d[:])

            # --- embedding gather (token-major): G[p, g, :] = E[tok[p, g], :]
            G = gpool.tile([P, NG, D], bf16, tag="g", name="g")
            scratch = dpool.tile([TWPAD, D], bf16, tag="scr", name="scr")
            for g in range(NG):
                nc.gpsimd.indirect_dma_start(
                    out=G[:, g, :],
                    out_offset=None,
                    in_=E_d[:],
                    in_offset=bass.IndirectOffsetOnAxis(
                        ap=tok_s[:, g : g + 1], axis=0
                    ),
                )
                # bounce group to DRAM scratch rows as soon as it lands
                nc.scalar.dma_start(scratch[g * P : (g + 1) * P, :], G[:, g, :])

            # two xbar transposes back to feature-major
            # GT[p, f, t] = E[tok[t], f*128+p]
            GT = gtpool.tile([P, DK, TWPAD], bf16, tag="gt", name="gt")
            HD = DK // 2 * P
            nc.scalar.dma_start_transpose(GT[:, : DK // 2, :], scratch[:, :HD])
            nc.scalar.dma_start_transpose(GT[:, DK // 2 :, :], scratch[:, HD:])

            # PE warmup: burn the HAM cold window on dummy matmuls while the
            # gather/transpose pipeline is in flight, so the real matmul
            # stream starts at full clock with no idle gap.
            warm = cpool.tile([P, T], bf16, tag="warm")
            nc.vector.memset(warm[:], 0.5)
            warm_ps = pspool.tile([P, T], f32, tag="ps", name="warm_ps")
            for _ in range(N_WARM):
                nc.tensor.matmul(
                    warm_ps[:], lhsT=warm[:, :P], rhs=warm[:],
                    start=True, stop=True,
                )

            b1_s = cpool.tile([P, DK], f32, tag="b1")
            nc.sync.dma_start(b1_s[:], b1_d[:])
            b2_s = cpool.tile([P, DK], f32, tag="b2")
            nc.sync.dma_start(b2_s[:], b2_d[:])
            bo_s = cpool.tile([P, NM], f32, tag="bo")
            nc.sync.dma_start(bo_s[:], bo_d[:])

            # --- MLP layer 1: h1^T = silu(W1^T x^T + b1) ---
            # k-outer loop: all 8 output blocks accumulate in parallel across
            # the 8 PSUM banks, so compute can start on the first GT half.
            w1_t = wpool.tile([P, KC * DK, D], bf16, tag="w1", name="w1")
            nc.sync.dma_start(w1_t[:], W1_d.rearrange("(k p) d -> p k d", p=P))
            h1 = [hpool.tile([P, T], bf16, tag=f"h1_{m}", name=f"h1_{m}") for m in range(DK)]
            ps1 = [pspool.tile([P, T], f32, tag="ps", name=f"ps1_{m}") for m in range(DK)]
            for k8 in range(DK):
                for i in range(KC):
                    for m in range(DK):
                        nc.tensor.matmul(
                            ps1[m][:],
                            lhsT=w1_t[:, i * DK + k8, m * P : (m + 1) * P],
                            rhs=GT[:, k8, i : i + T],
                            start=(k8 == 0 and i == 0),
                            stop=(k8 == DK - 1 and i == KC - 1),
                        )
            for m in range(DK):
                nc.scalar.activation(h1[m][:], ps1[m][:], AF.Silu, bias=b1_s[:, m : m + 1])

            # --- MLP layer 2: h2^T = silu(W2^T h1^T + b2) ---
            w2_t = wpool.tile([P, DK, D], bf16, tag="w2", name="w2")
            nc.sync.dma_start(w2_t[:], W2_d.rearrange("(k p) d -> p k d", p=P))
            h2 = [hpool.tile([P, T], bf16, tag=f"h2_{m}", name=f"h2_{m}") for m in range(DK)]
            for m in range(DK):
                ps = pspool.tile([P, T], f32, tag="ps")
                for k8 in range(DK):
                    nc.tensor.matmul(
                        ps[:],
                        lhsT=w2_t[:, k8, m * P : (m + 1) * P],
                        rhs=h1[k8][:],
                        start=(k8 == 0),
                        stop=(k8 == DK - 1),
                    )
                nc.scalar.activation(h2[m][:], ps[:], AF.Silu, bias=b2_s[:, m : m + 1])

            # --- vocab projection: logits^T = Wout^T h2^T + bout ---
            Wo_v = Wo_d.rearrange("(k p) v -> p k v", p=P)
            out_v = out_d.rearrange("(q p) t -> p q t", p=P)
            c0 = 0
            while c0 < VP:
                cols = min(MG, VP - c0)
                wos = wopool.tile([P, DK, MG], bf16, tag="wo", name=f"wo{c0}")
                nc.sync.dma_start(wos[:, :, :cols], Wo_v[:, :, c0 : c0 + cols])
                nmt = cols // P
                m = 0
                while m < nmt:
                    sb = min(SBATCH, nmt - m)
                    ot = opool.tile([P, SBATCH, T], bf16, tag="ot")
                    for j in range(sb):
                        ps = pspool.tile([P, T], f32, tag="ps")
                        for k8 in range(DK):
                            nc.tensor.matmul(
                                ps[:],
                                lhsT=wos[:, k8, (m + j) * P : (m + j + 1) * P],
                                rhs=h2[k8][:],
                                start=(k8 == 0),
                                stop=(k8 == DK - 1),
                            )
                        mi = (c0 + (m + j) * P) // P
                        nc.scalar.activation(
                            ot[:, j, :], ps[:], AF.Identity, bias=bo_s[:, mi : mi + 1]
                        )
                    q0 = (c0 + m * P) // P
                    nc.scalar.dma_start(
                        out_v[:, q0 : q0 + sb, :], ot[:, :sb, :]
                    )
                    m += sb
                c0 += cols

    nc.finalize()
    return nc


def _get_nc(V, D, KC, T, VP, MG):
    key = (V, D, KC, T, VP, MG)
    if key not in _nc_cache:
        _nc_cache[key] = _build(V, D, KC, T, VP, MG)
    return _nc_cache[key]


def _run(tokens, E, W1, b1, W2, b2, Wout, bout, V, D, KC, VP, MG, trace=False):
    """tokens: (S, B) int32.  Returns (S, B, V) f32 logits (and results obj)."""
    bf16 = ml_dtypes.bfloat16
    S, B = tokens.shape
    cpb = NCORES // B  # cores per batch column
    T = S // cpb
    DK = D // P
    TW = T + KC - 1
    NG = math.ceil(TW / P)
    TWPAD = NG * P
    NM = VP // P

    E_b = E.astype(bf16)
    W1_b = W1.astype(bf16)
    W2_b = W2.astype(bf16)
    Wo_b = np.zeros((D, VP), dtype=bf16)
    Wo_b[:, :V] = Wout.astype(bf16)
    b1t = np.ascontiguousarray(b1.reshape(DK, P).T.astype(np.float32))
    b2t = np.ascontiguousarray(b2.reshape(DK, P).T.astype(np.float32))
    bo_p = np.zeros(VP, dtype=np.float32)
    bo_p[:V] = bout
    bot = np.ascontiguousarray(bo_p.reshape(NM, P).T)

    nc = _get_nc(V, D, KC, T, VP, MG)

    in_maps = []
    for c in range(NCORES):
        b, chunk = divmod(c, cpb)
        s0 = chunk * T
        pad = np.zeros(TWPAD, dtype=np.int32)
        lo = max(0, s0 - (KC - 1))
        seg = tokens[lo : s0 + T, b]
        start = (KC - 1) - (s0 - lo)
        pad[start : start + seg.size] = seg
        tok2d = np.ascontiguousarray(pad.reshape(NG, P).T)
        in_maps.append(
            {
                "E": E_b,
                "W1": W1_b,
                "W2": W2_b,
                "Wo": Wo_b,
                "b1": b1t,
                "b2": b2t,
                "bo": bot,
                "toks": tok2d,
            }
        )

    kres = run_bass_kernel_spmd(nc, in_maps, list(range(NCORES)), trace=trace)
    res = kres.results

    out = np.empty((S, B, V), dtype=np.float32)
    for c in range(NCORES):
        b, chunk = divmod(c, cpb)
        s0 = chunk * T
        out[s0 : s0 + T, b, :] = res[c]["out"][:V, :].T.astype(np.float32)
    return out, kres


def kernel(**inputs):
    tokens = np.asarray(inputs["tokens_seq"]).astype(np.int32)
    E = np.asarray(inputs["E"], dtype=np.float32)
    W1 = np.asarray(inputs["W1"], dtype=np.float32)
    b1 = np.asarray(inputs["b1"], dtype=np.float32)
    W2 = np.asarray(inputs["W2"], dtype=np.float32)
    b2 = np.asarray(inputs["b2"], dtype=np.float32)
    Wout = np.asarray(inputs["Wout"], dtype=np.float32)
    bout = np.asarray(inputs["bout"], dtype=np.float32)
    out, _ = _run(
        tokens, E, W1, b1, W2, b2, Wout, bout,
        V=VOCAB, D=EMBED, KC=KGRAM, VP=VPAD, MG=MGROUP,
    )
    return out


# revision 10
# speedup vs baseline: 1.1939x; 1.1889x over previous
"""KGram MLP seq model (k-gram embedding lookup + 2-layer MLP + vocab projection)
on 8 Trainium2 NeuronCores.

Strategy: data-parallel over the S*B = 4096 token positions (512 rows/core,
cores 0-3 take batch 0, cores 4-7 take batch 1; each core owns a contiguous
span of 512 sequence positions of one batch column).  All weights are
replicated per core (uploaded as bf16).  Per core:

  1. indirect-DMA gather of the 640 (padded) embedding rows from E
     into a token-major [128, NG, D] tile (one DMA per 128-token group)
  2. bounce each group to a DRAM scratch [640, D] (row-major by position),
     then two big DRAM->SBUF xbar transposes into the feature-major layout
     GT[p, f, t] = E[tok[t], f*128+p]  ([128, DK, TWPAD] tile)
  3. h1^T = silu(W1^T x^T + b1) where the three K-blocks of x^T are just
     shifted column windows of GT (the k-gram windows overlap); k-outer
     loop accumulates all 8 output blocks across the 8 PSUM banks
  4. h2^T = silu(W2^T h1^T + b2)
  5. logits^T = Wout^T h2^T + bout, streamed over vocab in 1024-col groups
     (bf16 weights, f32 PSUM accumulate, bf16 output store; host upcasts)

PE warmup matmuls cover the gather/transpose window so the HAM clock never
drops; heavy DMA is split across the two HWDGE rings (sync: weights in,
scalar: logits out) with batched transfers (one DMA per weight group, one
store per 4 output tiles).

Host reassembles out[s, b, :] from the per-core logits^T shards.
"""

import math

import numpy as np
import ml_dtypes

import concourse.bass as bass
import concourse.mybir as mybir
import concourse.tile as tile
from concourse import bacc
from concourse.bass_utils import run_bass_kernel_spmd

P = 128
NCORES = 8

# Full-problem constants (hardcoded; kernel.py must be self-contained)
VOCAB = 50257
EMBED = 1024
SEQ = 2048
BATCH = 2
KGRAM = 3
VPAD = 50304  # 393 * 128
MGROUP = 1024  # vocab columns per Wout streaming group
SBATCH = 4    # output tiles per store DMA
N_WARM = 100  # FD-512 warmup matmuls covering the gather/transpose window

_nc_cache: dict = {}


def _build(V, D, KC, T, VP, MG):
    """Build the single-core Bass graph (SPMD: same graph on all cores)."""
    DK = D // P
    TW = T + KC - 1
    NG = math.ceil(TW / P)
    TWPAD = NG * P
    NM = VP // P
    f32 = mybir.dt.float32
    bf16 = mybir.dt.bfloat16
    i32 = mybir.dt.int32
    AF = mybir.ActivationFunctionType

    nc = bacc.Bacc()

    E_d = nc.declare_dram_parameter("E", [V, D], bf16, isOutput=False)
    W1_d = nc.declare_dram_parameter("W1", [KC * D, D], bf16, isOutput=False)
    W2_d = nc.declare_dram_parameter("W2", [D, D], bf16, isOutput=False)
    Wo_d = nc.declare_dram_parameter("Wo", [D, VP], bf16, isOutput=False)
    b1_d = nc.declare_dram_parameter("b1", [P, DK], f32, isOutput=False)
    b2_d = nc.declare_dram_parameter("b2", [P, DK], f32, isOutput=False)
    bo_d = nc.declare_dram_parameter("bo", [P, NM], f32, isOutput=False)
    tok_d = nc.declare_dram_parameter("toks", [P, NG], i32, isOutput=False)
    out_d = nc.declare_dram_parameter("out", [VP, T], bf16, isOutput=True)

    with tile.TileContext(nc) as tc:
        with (
            tc.tile_pool(name="const", bufs=1) as cpool,
            tc.tile_pool(name="gath", bufs=1) as gpool,
            tc.tile_pool(name="gt", bufs=1) as gtpool,
            tc.tile_pool(name="dram", bufs=1, space="DRAM") as dpool,
            tc.tile_pool(name="w", bufs=1) as wpool,
            tc.tile_pool(name="h", bufs=1) as hpool,
            tc.tile_pool(name="wo", bufs=2) as wopool,
            tc.tile_pool(name="ot", bufs=4) as opool,
            tc.tile_pool(name="ps", bufs=8, space="PSUM") as pspool,
        ):
            # token indices first so the gather starts immediately
            tok_s = cpool.tile([P, NG], i32, tag="tok")
            nc.sync.dma_start(tok_s[:], tok_d[:])

            # warm tile for PE warmup matmuls; memset issues on gpsimd before
            # the gathers (which wait on the token DMA anyway)
            warm = cpool.tile([P, T], bf16, tag="warm")
            nc.gpsimd.memset(warm[:], 0.5)

            # --- embedding gather (token-major): G[p, g, :] = E[tok[p, g], :]
            G = gpool.tile([P, NG, D], bf16, tag="g", name="g")
            scratch = dpool.tile([TWPAD, D], bf16, tag="scr", name="scr")
            for g in range(NG):
                nc.gpsimd.indirect_dma_start(
                    out=G[:, g, :],
                    out_offset=None,
                    in_=E_d[:],
                    in_offset=bass.IndirectOffsetOnAxis(
                        ap=tok_s[:, g : g + 1], axis=0
                    ),
                )
                # bounce group to DRAM scratch rows as soon as it lands
                nc.scalar.dma_start(scratch[g * P : (g + 1) * P, :], G[:, g, :])

            # one xbar transpose back to feature-major (contiguous source
            # rows -> fast path): GT[p, f, t] = E[tok[t], f*128+p]
            GT = gtpool.tile([P, DK, TWPAD], bf16, tag="gt", name="gt")
            nc.scalar.dma_start_transpose(GT[:], scratch[:])

            # PE warmup: burn the HAM cold window on dummy matmuls while the
            # gather/transpose pipeline is in flight, so the real matmul
            # stream starts at full clock with no idle gap.
            warm_ps = pspool.tile([P, T], f32, tag="ps", name="warm_ps")
            for _ in range(N_WARM):
                nc.tensor.matmul(
                    warm_ps[:], lhsT=warm[:, :P], rhs=warm[:],
                    start=True, stop=True,
                )

            b1_s = cpool.tile([P, DK], f32, tag="b1")
            nc.sync.dma_start(b1_s[:], b1_d[:])
            b2_s = cpool.tile([P, DK], f32, tag="b2")
            nc.sync.dma_start(b2_s[:], b2_d[:])
            bo_s = cpool.tile([P, NM], f32, tag="bo")
            nc.sync.dma_start(bo_s[:], bo_d[:])

            # --- MLP layer 1: h1^T = silu(W1^T x^T + b1) ---
            # k-outer loop: all 8 output blocks accumulate in parallel across
            # the 8 PSUM banks, so compute can start on the first GT half.
            w1_t = wpool.tile([P, KC * DK, D], bf16, tag="w1", name="w1")
            nc.sync.dma_start(w1_t[:], W1_d.rearrange("(k p) d -> p k d", p=P))
            h1 = [hpool.tile([P, T], bf16, tag=f"h1_{m}", name=f"h1_{m}") for m in range(DK)]
            ps1 = [pspool.tile([P, T], f32, tag="ps", name=f"ps1_{m}") for m in range(DK)]
            for k8 in range(DK):
                for i in range(KC):
                    for m in range(DK):
                        nc.tensor.matmul(
                            ps1[m][:],
                            lhsT=w1_t[:, i * DK + k8, m * P : (m + 1) * P],
                            rhs=GT[:, k8, i : i + T],
                            start=(k8 == 0 and i == 0),
                            stop=(k8 == DK - 1 and i == KC - 1),
                        )
            for m in range(DK):
                nc.scalar.activation(h1[m][:], ps1[m][:], AF.Silu, bias=b1_s[:, m : m + 1])

            # --- MLP layer 2: h2^T = silu(W2^T h1^T + b2) ---
            w2_t = wpool.tile([P, DK, D], bf16, tag="w2", name="w2")
            nc.sync.dma_start(w2_t[:], W2_d.rearrange("(k p) d -> p k d", p=P))
            h2 = [hpool.tile([P, T], bf16, tag=f"h2_{m}", name=f"h2_{m}") for m in range(DK)]
            for m in range(DK):
                ps = pspool.tile([P, T], f32, tag="ps")
                for k8 in range(DK):
                    nc.tensor.matmul(
                        ps[:],
                        lhsT=w2_t[:, k8, m * P : (m + 1) * P],
                        rhs=h1[k8][:],
                        start=(k8 == 0),
                        stop=(k8 == DK - 1),
                    )
                nc.scalar.activation(h2[m][:], ps[:], AF.Silu, bias=b2_s[:, m : m + 1])

            # --- vocab projection: logits^T = Wout^T h2^T + bout ---
            Wo_v = Wo_d.rearrange("(k p) v -> p k v", p=P)
            out_v = out_d.rearrange("(q p) t -> p q t", p=P)
            c0 = 0
            while c0 < VP:
                cols = min(MG, VP - c0)
                wos = wopool.tile([P, DK, MG], bf16, tag="wo", name=f"wo{c0}")
                nc.sync.dma_start(wos[:, :, :cols], Wo_v[:, :, c0 : c0 + cols])
                nmt = cols // P
                m = 0
                while m < nmt:
                    sb = min(SBATCH, nmt - m)
                    ot = opool.tile([P, SBATCH, T], bf16, tag="ot")
                    for j in range(sb):
                        ps = pspool.tile([P, T], f32, tag="ps")
                        for k8 in range(DK):
                            nc.tensor.matmul(
                                ps[:],
                                lhsT=wos[:, k8, (m + j) * P : (m + j + 1) * P],
                                rhs=h2[k8][:],
                                start=(k8 == 0),
                                stop=(k8 == DK - 1),
                            )
                        mi = (c0 + (m + j) * P) // P
                        nc.scalar.activation(
                            ot[:, j, :], ps[:], AF.Identity, bias=bo_s[:, mi : mi + 1]
                        )
                    q0 = (c0 + m * P) // P
                    nc.scalar.dma_start(
                        out_v[:, q0 : q0 + sb, :], ot[:, :sb, :]
                    )
                    m += sb
                c0 += cols

    nc.finalize()
    return nc


def _get_nc(V, D, KC, T, VP, MG):
    key = (V, D, KC, T, VP, MG)
    if key not in _nc_cache:
        _nc_cache[key] = _build(V, D, KC, T, VP, MG)
    return _nc_cache[key]


def _run(tokens, E, W1, b1, W2, b2, Wout, bout, V, D, KC, VP, MG, trace=False):
    """tokens: (S, B) int32.  Returns (S, B, V) f32 logits (and results obj)."""
    bf16 = ml_dtypes.bfloat16
    S, B = tokens.shape
    cpb = NCORES // B  # cores per batch column
    T = S // cpb
    DK = D // P
    TW = T + KC - 1
    NG = math.ceil(TW / P)
    TWPAD = NG * P
    NM = VP // P

    E_b = E.astype(bf16)
    W1_b = W1.astype(bf16)
    W2_b = W2.astype(bf16)
    Wo_b = np.zeros((D, VP), dtype=bf16)
    Wo_b[:, :V] = Wout.astype(bf16)
    b1t = np.ascontiguousarray(b1.reshape(DK, P).T.astype(np.float32))
    b2t = np.ascontiguousarray(b2.reshape(DK, P).T.astype(np.float32))
    bo_p = np.zeros(VP, dtype=np.float32)
    bo_p[:V] = bout
    bot = np.ascontiguousarray(bo_p.reshape(NM, P).T)

    nc = _get_nc(V, D, KC, T, VP, MG)

    in_maps = []
    for c in range(NCORES):
        b, chunk = divmod(c, cpb)
        s0 = chunk * T
        pad = np.zeros(TWPAD, dtype=np.int32)
        lo = max(0, s0 - (KC - 1))
        seg = tokens[lo : s0 + T, b]
        start = (KC - 1) - (s0 - lo)
        pad[start : start + seg.size] = seg
        tok2d = np.ascontiguousarray(pad.reshape(NG, P).T)
        in_maps.append(
            {
                "E": E_b,
                "W1": W1_b,
                "W2": W2_b,
                "Wo": Wo_b,
                "b1": b1t,
                "b2": b2t,
                "bo": bot,
                "toks": tok2d,
            }
        )

    kres = run_bass_kernel_spmd(nc, in_maps, list(range(NCORES)), trace=trace)
    res = kres.results

    out = np.empty((S, B, V), dtype=np.float32)
    for c in range(NCORES):
        b, chunk = divmod(c, cpb)
        s0 = chunk * T
        out[s0 : s0 + T, b, :] = res[c]["out"][:V, :].T.astype(np.float32)
    return out, kres


def kernel(**inputs):
    tokens = np.asarray(inputs["tokens_seq"]).astype(np.int32)
    E = np.asarray(inputs["E"], dtype=np.float32)
    W1 = np.asarray(inputs["W1"], dtype=np.float32)
    b1 = np.asarray(inputs["b1"], dtype=np.float32)
    W2 = np.asarray(inputs["W2"], dtype=np.float32)
    b2 = np.asarray(inputs["b2"], dtype=np.float32)
    Wout = np.asarray(inputs["Wout"], dtype=np.float32)
    bout = np.asarray(inputs["bout"], dtype=np.float32)
    out, _ = _run(
        tokens, E, W1, b1, W2, b2, Wout, bout,
        V=VOCAB, D=EMBED, KC=KGRAM, VP=VPAD, MG=MGROUP,
    )
    return out
